# revision 55
# baseline (speedup 1.0000x reference)
"""Multi-head GAT layer for Trainium2 — 8 heads sharded across 8 NeuronCores.

Per head h (N=4096 nodes, F=64 features):
    ltg   = graph @ W[h]                          [N, F]
    s     = ltg @ a_src,  d = ltg @ a_dst         [N]
    E     = leaky_relu(s[:, None] + d[None, :], 0.2)
    Alpha = softmax(E, axis=-1)
    out   = Alpha @ ltg

Algebraic core (as v1): with z = s_i + d_j, M_ij = [z >= 0],
    exp(leaky_relu(z)) = M e^{s_i} e^{d_j} + (1-M) e^{0.2 s_i} e^{0.2 d_j}
so softmax reduces to mask generation (DVE/ACT/Pool) + masked matmuls
(PE) over R = [R1|R2|v|v2] = [e^d.ltg | e^{.2d}.ltg | e^d | e^{.2d}],
with num_i/e^{.2 s_i} = r_i (M@R1)_i + (T2 - M@R2)_i, r = e^{.8 s}.

v2 restructure (vs the 141.6us v1):
  * host passes graph^T / W^T / a^T (pure layout) so the graph loads as
    4 large DMAs and the 32 PE transposes + their DVE/ACT drains vanish.
  * ACT-generated masks use Sign (+-1, in every ACT table set) instead
    of saturated Sigmoid: zero ACT_TABLE_LOAD thrash against the exps.
    +-1 blocks store R/2 (exp bias -ln2); the constant column error
    C = sum_ACT T_b/2 is added back during the psum->SBUF drain via a
    broadcast C tensor, so the epilogue algebra is unchanged.
  * PSUM: 2-bank accumulators (2 i-blocks/sup, 16 sups) double-buffered
    -> no per-sup drain stall; setup keeps 4 banks for projections and
    the s-broadcast.
  * masks batched [128, 512] (2 sups per instruction) to amortize the
    fixed per-instruction cost on DVE (~105ns) and ACT (~293ns).
  * out rows staged [128, 256] and DMA'd per 4 blocks (8 descriptors
    instead of 32 dma_starts).

Heads are fully independent: core h computes head h; no collectives.
"""

import math
import os
from contextlib import ExitStack

import numpy as np

N, F_IN, F, H = 4096, 64, 64, 8
P = 128
NB = N // P           # 32 node blocks (j)
NG = NB // 4          # 8 groups of 4 blocks
ISUP = 2              # i-blocks per PSUM accumulator (2 banks)
NSUP = NB // ISUP     # 16 sup iterations
SB = int(os.environ.get("GAT_SB", "2"))  # sups per mask batch
NBATCH = NSUP // SB   # mask batches
RC = 130              # R columns per j-block: R1(64) | v | R2(64) | v2
                      # (r-scaled part contiguous at 0:65 for the drain)
LN2 = math.log(2.0)

# mask-engine split in GROUPS of 4 blocks: (DVE, Pool, ACT).
# Pool is_ge is software-emulated at ~8.7us per [128,512] tile and starves
# DVE via SBUF port contention -> keep GP=0.
GV = int(os.environ.get("GAT_GV", "5"))
GP = int(os.environ.get("GAT_GP", "0"))
GA = int(os.environ.get("GAT_GA", "3"))
assert GV + GP + GA == NG
_CACHE = {}


def _build():
    import concourse.bass as bass  # noqa: F401
    import concourse.mybir as mybir
    import concourse.tile as tile
    from concourse import bacc

    dt = mybir.dt
    f32 = dt.float32
    f16 = dt.float16
    bf16 = dt.bfloat16
    Alu = mybir.AluOpType
    Act = mybir.ActivationFunctionType

    mask_eng = (["v"] * (4 * GV) + ["p"] * (4 * GP) + ["a"] * (4 * GA))

    nc = bacc.Bacc("TRN2", debug=False, num_devices=H)
    gt_d = nc.dram_tensor("gt", [F_IN, N], f32, kind="ExternalInput").ap()
    w_d = nc.dram_tensor("w", [F_IN, F], f32, kind="ExternalInput").ap()
    wt_d = nc.dram_tensor("wt", [F, F_IN], f32, kind="ExternalInput").ap()
    a2_d = nc.dram_tensor("a2", [F, 2], f32, kind="ExternalInput").ap()
    out_d = nc.dram_tensor("out", [N, F], f32, kind="ExternalOutput").ap()

    with tile.TileContext(nc) as tc, ExitStack() as ctx:
        persist = ctx.enter_context(tc.tile_pool(name="persist", bufs=1))
        sps = ctx.enter_context(tc.tile_pool(name="sps", bufs=3, space="PSUM"))
        bcp = ctx.enter_context(tc.tile_pool(name="bcp", bufs=1, space="PSUM"))
        accp = ctx.enter_context(tc.tile_pool(name="acc", bufs=2, space="PSUM"))
        mp = ctx.enter_context(tc.tile_pool(name="mask", bufs=2))
        ep = ctx.enter_context(tc.tile_pool(name="ep", bufs=6))
        osp = ctx.enter_context(tc.tile_pool(name="osp", bufs=2))

        # ---- input DMAs ----
        gsp = ctx.enter_context(tc.tile_pool(name="gsp", bufs=2))
        gstages = []
        gst = gsp.tile([F_IN, 512], f32, tag="g0", name="gst0", bufs=1)
        nc.sync.dma_start(gst[:], gt_d[:, 0:512])
        gstages.append(gst)
        wf = persist.tile([F_IN, F], f32)
        nc.sync.dma_start(wf[:], w_d[:])
        wt_sb = persist.tile([F, F_IN], f32)
        nc.sync.dma_start(wt_sb[:], wt_d[:])
        a2_sb = persist.tile([F, 2], f32)
        nc.sync.dma_start(a2_sb[:], a2_d[:])
        gst = gsp.tile([F_IN, 512], f32, tag="g1", name="gst1", bufs=1)
        nc.sync.dma_start(gst[:], gt_d[:, 512:1024])
        gstages.append(gst)
        for c in range(2, 8, 2):
            gst = gsp.tile([F_IN, 1024], f32, tag="gst", name="gst")
            nc.sync.dma_start(gst[:], gt_d[:, 512 * c:512 * (c + 2)])
            gstages.append(gst)
        # view list: chunk c (512 cols) -> (tile, col offset)
        gviews = [(gstages[0], 0), (gstages[1], 0)]
        for c in range(2, 8):
            gviews.append((gstages[2 + (c - 2) // 2], 512 * (c % 2)))

        ones_row_bf = persist.tile([1, P], bf16)
        nc.vector.memset(ones_row_bf[:], 1.0)
        ones_col_bf = persist.tile([P, 1], bf16)
        nc.gpsimd.memset(ones_col_bf[:], 1.0)
        ones_row_f = persist.tile([1, P], f32)
        nc.vector.memset(ones_row_f[:], 1.0)
        negln2 = persist.tile([P, 1], f32)
        nc.vector.memset(negln2[:], -LN2)

        # fused fp16 [W | w_s | w_d]; w_sd = W @ a2 via host-provided W^T
        w16 = persist.tile([F_IN, F + 2], f16)
        nc.gpsimd.tensor_copy(w16[:, 0:F], wf[:])
        wsd_ps = sps.tile([F_IN, 2], f32, tag="pj", name="wsd_ps")
        nc.tensor.matmul(wsd_ps[:], wt_sb[:], a2_sb[:])
        nc.scalar.copy(w16[:, F:F + 2], wsd_ps[:])

        gT16 = persist.tile([F_IN, N], f16)
        sdrow = persist.tile([2, N], bf16)
        s_rep = persist.tile([P, N], bf16)
        r_all = persist.tile([P, RC * NB], bf16)
        r_v = r_all.rearrange("p (b c) -> p b c", c=RC)
        eps_all = persist.tile([P, RC * NB], f32)
        eps_v = eps_all.rearrange("p (b c) -> p b c", c=RC)
        scol = persist.tile([P, NB], f32)
        dcol = persist.tile([P, NB], f32)
        negd = persist.tile([P, NB], f32)
        vcol = persist.tile([P, NB], f32)
        v2col = persist.tile([P, NB], f32)
        rcol = persist.tile([P, NB], f32)
        crep = persist.tile([P, 65], f32)        # [R2 totals | v2 total] bcast
        cacc = persist.tile([1, 65], f32)
        chl = persist.tile([2, RC], bf16)        # C as bf16 hi/lo rows
        clo_f = persist.tile([1, RC], f32)
        clo_bf = persist.tile([1, RC], bf16)
        ones2 = persist.tile([2, P], bf16)
        nc.vector.memset(ones2[:], 1.0)

        mask_tiles = {}
        acc_tiles = {}

        def get_acc(sup):
            if sup not in acc_tiles:
                acc_tiles[sup] = accp.tile([P, 1024], f32, tag="acc",
                                           name=f"acc{sup}")
            return acc_tiles[sup]

        def emit_mask(batch, b):
            eng = mask_eng[b]
            i0 = batch * SB * ISUP * P
            mt = mp.tile([P, SB * ISUP * P], bf16, tag=f"m{b}", name=f"mask{b}")
            src = s_rep[:, i0:i0 + SB * ISUP * P]
            if eng == "a":
                nc.scalar.activation(mt[:], src, Act.Sign, bias=dcol[:, b:b + 1])
            elif eng == "v":
                nc.vector.tensor_scalar(mt[:], src, negd[:, b:b + 1], None,
                                        op0=Alu.is_ge)
            else:
                nc.gpsimd.tensor_scalar(mt[:], src, negd[:, b:b + 1], None,
                                        op0=Alu.is_ge)
            mask_tiles[(batch, b)] = mt

        def any_copy(eng, dst, src):
            if eng is nc.scalar:
                eng.copy(dst, src)
            else:
                eng.tensor_copy(dst, src)

        # ---- graph cast + s-row broadcast, per 512-col chunk ----
        cast_eng = [nc.vector, nc.gpsimd, nc.vector, nc.gpsimd,
                    nc.scalar, nc.gpsimd, nc.vector, nc.gpsimd]
        # ---- per-chunk fused setup: cast + s-broadcast + projection group.
        # Group g needs exactly gT16 chunk g, and allocating the pj-ring
        # tiles in time order keeps the ring from gating early groups on
        # late DMA chunks.
        for g in range(8):
            gtile, goff = gviews[g]
            any_copy(cast_eng[g], gT16[:, 512 * g:512 * (g + 1)],
                     gtile[:, goff:goff + 512])
            srow_ps = sps.tile([2, 512], f32, tag="pj", name="srow_ps")
            nc.tensor.matmul(srow_ps[:], w16[:, F:F + 2],
                             gT16[:, 512 * g:512 * (g + 1)])
            nc.vector.tensor_copy(sdrow[:, 512 * g:512 * (g + 1)],
                                  srow_ps[:])
            bc_ps = bcp.tile([P, 512], f32, tag="bc", name="bc_ps")
            nc.tensor.matmul(bc_ps[:],
                             ones_row_bf[:], sdrow[0:1, 512 * g:512 * (g + 1)])
            seng = nc.vector if g % 2 == 0 else nc.scalar
            any_copy(seng, s_rep[:, 512 * g:512 * (g + 1)], bc_ps[:])

            # projection + R construction for this group's 4 blocks
            is_a = mask_eng[4 * g] == "a"
            prj = sps.tile([P, 4 * 66], f32, tag="pj", name="prj")
            prj_v = prj.rearrange("p (b c) -> p b c", c=66)
            for k in range(4):
                b = 4 * g + k
                nc.tensor.matmul(prj[:, 66 * k:66 * (k + 1)],
                                 gT16[:, P * b:P * (b + 1)], w16[:])
            bsl = slice(4 * g, 4 * g + 4)
            d_src = prj_v[:, :, 65]
            s_src = prj_v[:, :, 64]
            nc.vector.tensor_copy(scol[:, bsl], s_src)
            nc.vector.tensor_copy(dcol[:, bsl], d_src)
            nc.vector.tensor_scalar(negd[:, bsl], d_src, -1.0, None,
                                    op0=Alu.mult)
            # v = e^d, v2 = e^{0.2 d} (x1/2 for +-1 blocks)
            bias = negln2[:] if is_a else 0.0
            nc.scalar.activation(vcol[:, bsl], dcol[:, bsl], Act.Exp,
                                 bias=bias)
            nc.scalar.activation(v2col[:, bsl], dcol[:, bsl], Act.Exp,
                                 scale=0.2, bias=bias)
            for k in range(4):
                b = 4 * g + k
                r0 = RC * b
                nc.vector.tensor_scalar(r_all[:, r0:r0 + F], prj_v[:, k, 0:F],
                                        vcol[:, b:b + 1], None, op0=Alu.mult)
                nc.vector.tensor_tensor(
                    r_all[:, r0 + F + 1:r0 + 2 * F + 1],
                    prj_v[:, k, 0:F],
                    v2col[:, b:b + 1].to_broadcast([P, F]),
                    op=Alu.mult)
            nc.gpsimd.tensor_copy(r_v[:, bsl, F], vcol[:, bsl])
            nc.gpsimd.tensor_copy(r_v[:, bsl, 129], v2col[:, bsl])
            # batch-0 masks + sup-0 accumulation for this group's blocks
            acc0 = get_acc(0)
            for k in range(4):
                b = 4 * g + k
                emit_mask(0, b)
                for t in range(ISUP):
                    nc.tensor.matmul(acc0[:, 512 * t:512 * t + RC],
                                     mask_tiles[(0, b)][:, t * P:(t + 1) * P],
                                     r_v[:, b, 0:RC],
                                     start=(b == 0),
                                     stop=(b == NB - 1 and GA == 0))

        # ---- rcol + column-total bursts + broadcast of [C | T2true] ----
        nc.scalar.activation(rcol[:], scol[:], Act.Exp, scale=0.8)
        vp_blocks = [b for b in range(NB) if mask_eng[b] != "a"]
        a_blocks = [b for b in range(NB) if mask_eng[b] == "a"]
        bd_ps = sps.tile([1, RC], f32, tag="pj", name="bd_ps")
        for i, b in enumerate(vp_blocks):
            nc.tensor.matmul(bd_ps[:], ones_col_bf[:], r_v[:, b, 0:RC],
                             start=(i == 0), stop=(i == len(vp_blocks) - 1))
        if a_blocks:
            ba_ps = sps.tile([1, RC], f32, tag="pj", name="ba_ps")
            for i, b in enumerate(a_blocks):
                nc.tensor.matmul(ba_ps[:], ones_col_bf[:], r_v[:, b, 0:RC],
                                 start=(i == 0), stop=(i == len(a_blocks) - 1))
            # C -> bf16 hi/lo rows for the per-bank K=2 correction matmul
            # (row 1 is written via a tiny SBUF->SBUF DMA: engines cannot
            # address a tile starting at partition 1)
            nc.vector.tensor_copy(chl[0:1, :], ba_ps[:])
            nc.vector.tensor_tensor(clo_f[:], ba_ps[:], chl[0:1, :],
                                    op=Alu.subtract)
            nc.vector.tensor_copy(clo_bf[:], clo_f[:])
            nc.sync.dma_start(chl[1:2, :], clo_bf[:])
            bdt2 = persist.tile([1, 65], f32)
            nc.vector.tensor_copy(bdt2[:], bd_ps[:, F + 1:RC])
            nc.vector.scalar_tensor_tensor(cacc[:], ba_ps[:, F + 1:RC],
                                           2.0, bdt2[:],
                                           op0=Alu.mult, op1=Alu.add)
        else:
            nc.vector.tensor_copy(cacc[:], bd_ps[:, F + 1:RC])
        crep_ps = sps.tile([P, 65], f32, tag="pj", name="crep_ps")
        nc.tensor.matmul(crep_ps[:], ones_row_f[:], cacc[:])
        nc.scalar.copy(crep[:], crep_ps[:])

        # Striped block order for mask emission + chain consumption: each
        # sup's emission quota mixes DVE and ACT blocks proportionally so
        # both engines generate masks concurrently instead of in phases.
        vblocks = [b for b in range(NB) if mask_eng[b] == "v"]
        pblocks = [b for b in range(NB) if mask_eng[b] == "p"]
        ablocks = [b for b in range(NB) if mask_eng[b] == "a"]
        ORDER = []
        for h in range(SB):
            for cls in (vblocks, pblocks, ablocks):
                n0 = len(cls) * h // SB
                n1 = len(cls) * (h + 1) // SB
                ORDER.extend(cls[n0:n1])
        assert sorted(ORDER) == list(range(NB))

        # ---- epilogue ----
        late_pend = []

        def late_phase(lsup):
            """num/den algebra + out DMA for i-blocks 2*lsup, 2*lsup+1.
            eps[0:65] is already r-scaled by the drain; everything here is
            a plain Pool tensor_tensor (ACT/DVE stay free for masks)."""
            i0 = ISUP * lsup
            dd = ep.tile([P, ISUP], f32, tag="dd", name="dd")
            nc.gpsimd.tensor_tensor(dd[:],
                                    crep[:, 64:65].to_broadcast([P, ISUP]),
                                    eps_v[:, i0:i0 + ISUP, 129], op=Alu.subtract)
            den = ep.tile([P, ISUP], f32, tag="den", name="den")
            nc.gpsimd.tensor_tensor(den[:], eps_v[:, i0:i0 + ISUP, F], dd[:],
                                    op=Alu.add)
            rden = ep.tile([P, ISUP], f32, tag="rden", name="rden")
            nc.vector.reciprocal(rden[:], den[:])
            ost = osp.tile([P, ISUP * F], f32, tag="ost", name="ost")
            for t in range(ISUP):
                i = i0 + t
                e0 = RC * i
                B = ep.tile([P, F], f32, tag="nB", name="nB")
                nc.gpsimd.tensor_tensor(B[:], crep[:, 0:F],
                                        eps_all[:, e0 + F + 1:e0 + 2 * F + 1],
                                        op=Alu.subtract)
                num = ep.tile([P, F], f32, tag="num", name="num")
                nc.gpsimd.tensor_tensor(num[:], eps_all[:, e0:e0 + F], B[:],
                                        op=Alu.add)
                nc.gpsimd.tensor_tensor(ost[:, F * t:F * (t + 1)], num[:],
                                        rden[:, t:t + 1].to_broadcast([P, F]),
                                        op=Alu.mult)
            nc.sync.dma_start(
                out_d.rearrange("(c p) f -> p c f", p=P)[:, ISUP * lsup:ISUP * (lsup + 1), :],
                ost.rearrange("p (c f) -> p c f", f=F))

        # ---- main loop over sups (sup 0's matmuls were emitted in setup) ----
        for sup in range(NSUP):
            batch = sup // SB
            nb_ = batch + 1
            if nb_ < NBATCH:
                per = NB // SB
                half = sup % SB
                for b in ORDER[per * half:per * (half + 1)]:
                    emit_mask(nb_, b)
            acc = get_acc(sup)
            if sup > 0:
                off = (sup % SB) * ISUP * P
                for bi, b in enumerate(ORDER):
                    mt = mask_tiles[(batch, b)]
                    for t in range(ISUP):
                        nc.tensor.matmul(acc[:, 512 * t:512 * t + RC],
                                         mt[:, off + t * P:off + (t + 1) * P],
                                         r_v[:, b, 0:RC],
                                         start=(bi == 0),
                                         stop=(bi == NB - 1 and GA == 0))
            if GA > 0:
                # add the +-1-block column correction C into each bank
                for t in range(ISUP):
                    nc.tensor.matmul(acc[:, 512 * t:512 * t + RC],
                                     ones2[:], chl[:],
                                     start=False, stop=True)
            if sup % SB == SB - 1:
                for b in range(NB):
                    mask_tiles.pop((batch, b))
            for t in range(ISUP):
                i = sup * ISUP + t
                # r-scale R1|v during the drain; plain-copy R2|v2
                nc.vector.tensor_scalar(eps_all[:, RC * i:RC * i + F + 1],
                                        acc[:, 512 * t:512 * t + F + 1],
                                        rcol[:, i:i + 1], None, op0=Alu.mult)
                nc.vector.tensor_copy(eps_all[:, RC * i + F + 1:RC * (i + 1)],
                                      acc[:, 512 * t + F + 1:512 * t + RC])
            late_pend.append(sup)
            if len(late_pend) > 1:
                late_phase(late_pend.pop(0))
        for pr in late_pend:
            late_phase(pr)

    nc.compile()
    return nc


def _get_nc():
    if "nc" not in _CACHE:
        _CACHE["nc"] = _build()
    return _CACHE["nc"]


def kernel(graph, W, a):
    from concourse.bass_utils import run_bass_kernel_spmd

    graph = np.asarray(graph, dtype=np.float32)
    W = np.asarray(W, dtype=np.float32)
    a = np.asarray(a, dtype=np.float32)
    gt = np.ascontiguousarray(graph.T)

    nc = _get_nc()
    in_maps = [
        {
            "gt": gt,
            "w": np.ascontiguousarray(W[h]),
            "wt": np.ascontiguousarray(W[h].T),
            "a2": np.ascontiguousarray(a[h].reshape(2, F).T),
        }
        for h in range(H)
    ]
    trace = bool(int(os.environ.get("GAT_TRACE", "0")))
    res = run_bass_kernel_spmd(nc, in_maps, core_ids=list(range(H)), trace=trace)
    _CACHE["last_result"] = res
    return np.stack([res.results[h]["out"] for h in range(H)], axis=0)


# revision 56
# speedup vs baseline: 1.1758x; 1.1758x over previous
"""Multi-head GAT layer for Trainium2 — 8 heads sharded across 8 NeuronCores.

Per head h (N=4096 nodes, F=64 features):
    ltg   = graph @ W[h]                          [N, F]
    s     = ltg @ a_src,  d = ltg @ a_dst         [N]
    E     = leaky_relu(s[:, None] + d[None, :], 0.2)
    Alpha = softmax(E, axis=-1)
    out   = Alpha @ ltg

Algebraic core (as v1): with z = s_i + d_j, M_ij = [z >= 0],
    exp(leaky_relu(z)) = M e^{s_i} e^{d_j} + (1-M) e^{0.2 s_i} e^{0.2 d_j}
so softmax reduces to mask generation (DVE/ACT/Pool) + masked matmuls
(PE) over R = [R1|R2|v|v2] = [e^d.ltg | e^{.2d}.ltg | e^d | e^{.2d}],
with num_i/e^{.2 s_i} = r_i (M@R1)_i + (T2 - M@R2)_i, r = e^{.8 s}.

v2 restructure (vs the 141.6us v1):
  * host passes graph^T / W^T / a^T (pure layout) so the graph loads as
    4 large DMAs and the 32 PE transposes + their DVE/ACT drains vanish.
  * ACT-generated masks use Sign (+-1, in every ACT table set) instead
    of saturated Sigmoid: zero ACT_TABLE_LOAD thrash against the exps.
    +-1 blocks store R/2 (exp bias -ln2); the constant column error
    C = sum_ACT T_b/2 is added back during the psum->SBUF drain via a
    broadcast C tensor, so the epilogue algebra is unchanged.
  * PSUM: 2-bank accumulators (2 i-blocks/sup, 16 sups) double-buffered
    -> no per-sup drain stall; setup keeps 4 banks for projections and
    the s-broadcast.
  * masks batched [128, 512] (2 sups per instruction) to amortize the
    fixed per-instruction cost on DVE (~105ns) and ACT (~293ns).
  * out rows staged [128, 256] and DMA'd per 4 blocks (8 descriptors
    instead of 32 dma_starts).

Heads are fully independent: core h computes head h; no collectives.
"""

import math
import os
from contextlib import ExitStack

import numpy as np

N, F_IN, F, H = 4096, 64, 64, 8
P = 128
NB = N // P           # 32 node blocks (j)
NG = NB // 4          # 8 groups of 4 blocks
ISUP = 2              # i-blocks per PSUM accumulator (2 banks)
NSUP = NB // ISUP     # 16 sup iterations
SB = int(os.environ.get("GAT_SB", "2"))  # sups per mask batch
NBATCH = NSUP // SB   # mask batches
RC = 130              # R columns per j-block: R1(64) | v | R2(64) | v2
                      # (r-scaled part contiguous at 0:65 for the drain)
LN2 = math.log(2.0)

# mask-engine split in GROUPS of 4 blocks: (DVE, Pool, ACT).
# Pool is_ge is software-emulated at ~8.7us per [128,512] tile and starves
# DVE via SBUF port contention -> keep GP=0.
GV = int(os.environ.get("GAT_GV", "5"))
GP = int(os.environ.get("GAT_GP", "0"))
GA = int(os.environ.get("GAT_GA", "3"))
assert GV + GP + GA == NG
_CACHE = {}


def _build():
    import concourse.bass as bass  # noqa: F401
    import concourse.mybir as mybir
    import concourse.tile as tile
    from concourse import bacc

    dt = mybir.dt
    f32 = dt.float32
    f16 = dt.float16
    bf16 = dt.bfloat16
    Alu = mybir.AluOpType
    Act = mybir.ActivationFunctionType

    mask_eng = (["v"] * (4 * GV) + ["p"] * (4 * GP) + ["a"] * (4 * GA))

    nc = bacc.Bacc("TRN2", debug=False, num_devices=H)
    gt_d = nc.dram_tensor("gt", [F_IN, N], f32, kind="ExternalInput").ap()
    w_d = nc.dram_tensor("w", [F_IN, F], f32, kind="ExternalInput").ap()
    wt_d = nc.dram_tensor("wt", [F, F_IN], f32, kind="ExternalInput").ap()
    a2_d = nc.dram_tensor("a2", [F, 2], f32, kind="ExternalInput").ap()
    out_d = nc.dram_tensor("out", [N, F], f32, kind="ExternalOutput").ap()

    with tile.TileContext(nc) as tc, ExitStack() as ctx:
        persist = ctx.enter_context(tc.tile_pool(name="persist", bufs=1))
        sps = ctx.enter_context(tc.tile_pool(name="sps", bufs=3, space="PSUM"))
        bcp = ctx.enter_context(tc.tile_pool(name="bcp", bufs=1, space="PSUM"))
        accp = ctx.enter_context(tc.tile_pool(name="acc", bufs=2, space="PSUM"))
        mp = ctx.enter_context(tc.tile_pool(name="mask", bufs=2))
        ep = ctx.enter_context(tc.tile_pool(name="ep", bufs=6))
        osp = ctx.enter_context(tc.tile_pool(name="osp", bufs=2))

        # ---- input DMAs ----
        gsp = ctx.enter_context(tc.tile_pool(name="gsp", bufs=2))
        gstages = []
        gst = gsp.tile([F_IN, 512], f32, tag="g0", name="gst0", bufs=1)
        nc.sync.dma_start(gst[:], gt_d[:, 0:512])
        gstages.append(gst)
        wf = persist.tile([F_IN, F], f32)
        nc.sync.dma_start(wf[:], w_d[:])
        wt_sb = persist.tile([F, F_IN], f32)
        nc.sync.dma_start(wt_sb[:], wt_d[:])
        a2_sb = persist.tile([F, 2], f32)
        nc.sync.dma_start(a2_sb[:], a2_d[:])
        gst = gsp.tile([F_IN, 512], f32, tag="g1", name="gst1", bufs=1)
        nc.sync.dma_start(gst[:], gt_d[:, 512:1024])
        gstages.append(gst)
        for c in range(2, 8, 2):
            gst = gsp.tile([F_IN, 1024], f32, tag="gst", name="gst")
            nc.sync.dma_start(gst[:], gt_d[:, 512 * c:512 * (c + 2)])
            gstages.append(gst)
        # view list: chunk c (512 cols) -> (tile, col offset)
        gviews = [(gstages[0], 0), (gstages[1], 0)]
        for c in range(2, 8):
            gviews.append((gstages[2 + (c - 2) // 2], 512 * (c % 2)))

        ones_row_bf = persist.tile([1, P], bf16)
        nc.vector.memset(ones_row_bf[:], 1.0)
        ones_col_bf = persist.tile([P, 1], bf16)
        nc.gpsimd.memset(ones_col_bf[:], 1.0)
        ones_row_f = persist.tile([1, P], f32)
        nc.vector.memset(ones_row_f[:], 1.0)
        negln2 = persist.tile([P, 1], f32)
        nc.vector.memset(negln2[:], -LN2)

        # fused fp16 [W | w_s | w_d]; w_sd = W @ a2 via host-provided W^T
        w16 = persist.tile([F_IN, F + 2], f16)
        nc.gpsimd.tensor_copy(w16[:, 0:F], wf[:])
        wsd_ps = sps.tile([F_IN, 2], f32, tag="pj", name="wsd_ps")
        nc.tensor.matmul(wsd_ps[:], wt_sb[:], a2_sb[:])
        nc.scalar.copy(w16[:, F:F + 2], wsd_ps[:])

        gT16 = persist.tile([F_IN, N], f16)
        sdrow = persist.tile([2, N], bf16)
        s_rep = persist.tile([P, N], bf16)
        r_all = persist.tile([P, RC * NB], bf16)
        r_v = r_all.rearrange("p (b c) -> p b c", c=RC)
        eps_all = persist.tile([P, RC * NB], f32)
        eps_v = eps_all.rearrange("p (b c) -> p b c", c=RC)
        scol = persist.tile([P, NB], f32)
        dcol = persist.tile([P, NB], f32)
        negd = persist.tile([P, NB], f32)
        vcol = persist.tile([P, NB], f32)
        v2col = persist.tile([P, NB], f32)
        rcol = persist.tile([P, NB], f32)
        crep = persist.tile([P, 65], f32)        # [R2 totals | v2 total] bcast
        cacc = persist.tile([1, 65], f32)
        chl = persist.tile([2, RC], bf16)        # C as bf16 hi/lo rows
        clo_f = persist.tile([1, RC], f32)
        clo_bf = persist.tile([1, RC], bf16)
        ones2 = persist.tile([2, P], bf16)
        nc.vector.memset(ones2[:], 1.0)

        mask_tiles = {}
        acc_tiles = {}

        def get_acc(sup):
            if sup not in acc_tiles:
                acc_tiles[sup] = accp.tile([P, 1024], f32, tag="acc",
                                           name=f"acc{sup}")
            return acc_tiles[sup]

        def emit_mask(batch, b):
            eng = mask_eng[b]
            i0 = batch * SB * ISUP * P
            mt = mp.tile([P, SB * ISUP * P], bf16, tag=f"m{b}", name=f"mask{b}")
            src = s_rep[:, i0:i0 + SB * ISUP * P]
            if eng == "a":
                nc.scalar.activation(mt[:], src, Act.Sign, bias=dcol[:, b:b + 1])
            elif eng == "v":
                nc.vector.tensor_scalar(mt[:], src, negd[:, b:b + 1], None,
                                        op0=Alu.is_ge)
            else:
                nc.gpsimd.tensor_scalar(mt[:], src, negd[:, b:b + 1], None,
                                        op0=Alu.is_ge)
            mask_tiles[(batch, b)] = mt

        def any_copy(eng, dst, src):
            if eng is nc.scalar:
                eng.copy(dst, src)
            else:
                eng.tensor_copy(dst, src)

        # ---- graph cast + s-row broadcast, per 512-col chunk ----
        cast_eng = [nc.vector, nc.gpsimd, nc.gpsimd, nc.gpsimd,
                    nc.scalar, nc.gpsimd, nc.gpsimd, nc.gpsimd]
        # ---- per-chunk fused setup: cast + s-broadcast + projection group.
        # Group g needs exactly gT16 chunk g, and allocating the pj-ring
        # tiles in time order keeps the ring from gating early groups on
        # late DMA chunks.
        for g in range(8):
            gtile, goff = gviews[g]
            any_copy(cast_eng[g], gT16[:, 512 * g:512 * (g + 1)],
                     gtile[:, goff:goff + 512])
            srow_ps = sps.tile([2, 512], f32, tag="pj", name="srow_ps")
            nc.tensor.matmul(srow_ps[:], w16[:, F:F + 2],
                             gT16[:, 512 * g:512 * (g + 1)])
            nc.vector.tensor_copy(sdrow[:, 512 * g:512 * (g + 1)],
                                  srow_ps[:])
            bc_ps = bcp.tile([P, 512], f32, tag="bc", name="bc_ps")
            nc.tensor.matmul(bc_ps[:],
                             ones_row_bf[:], sdrow[0:1, 512 * g:512 * (g + 1)])
            seng = nc.vector if g % 2 == 0 else nc.scalar
            any_copy(seng, s_rep[:, 512 * g:512 * (g + 1)], bc_ps[:])

            # projection + R construction for this group's 4 blocks
            is_a = mask_eng[4 * g] == "a"
            prj = sps.tile([P, 4 * 66], f32, tag="pj", name="prj")
            prj_v = prj.rearrange("p (b c) -> p b c", c=66)
            for k in range(4):
                b = 4 * g + k
                nc.tensor.matmul(prj[:, 66 * k:66 * (k + 1)],
                                 gT16[:, P * b:P * (b + 1)], w16[:])
            bsl = slice(4 * g, 4 * g + 4)
            d_src = prj_v[:, :, 65]
            s_src = prj_v[:, :, 64]
            nc.vector.tensor_copy(scol[:, bsl], s_src)
            nc.vector.tensor_copy(dcol[:, bsl], d_src)
            nc.vector.tensor_scalar(negd[:, bsl], d_src, -1.0, None,
                                    op0=Alu.mult)
            # v = e^d, v2 = e^{0.2 d} (x1/2 for +-1 blocks)
            bias = negln2[:] if is_a else 0.0
            nc.scalar.activation(vcol[:, bsl], dcol[:, bsl], Act.Exp,
                                 bias=bias)
            nc.scalar.activation(v2col[:, bsl], dcol[:, bsl], Act.Exp,
                                 scale=0.2, bias=bias)
            for k in range(4):
                b = 4 * g + k
                r0 = RC * b
                nc.vector.tensor_scalar(r_all[:, r0:r0 + F], prj_v[:, k, 0:F],
                                        vcol[:, b:b + 1], None, op0=Alu.mult)
                nc.vector.tensor_tensor(
                    r_all[:, r0 + F + 1:r0 + 2 * F + 1],
                    prj_v[:, k, 0:F],
                    v2col[:, b:b + 1].to_broadcast([P, F]),
                    op=Alu.mult)
            nc.gpsimd.tensor_copy(r_v[:, bsl, F], vcol[:, bsl])
            nc.gpsimd.tensor_copy(r_v[:, bsl, 129], v2col[:, bsl])
            # batch-0 masks + sup-0 accumulation for this group's blocks
            acc0 = get_acc(0)
            for k in range(4):
                b = 4 * g + k
                emit_mask(0, b)
                for t in range(ISUP):
                    nc.tensor.matmul(acc0[:, 512 * t:512 * t + RC],
                                     mask_tiles[(0, b)][:, t * P:(t + 1) * P],
                                     r_v[:, b, 0:RC],
                                     start=(b == 0),
                                     stop=(b == NB - 1 and GA == 0))

        # ---- rcol + column-total bursts + broadcast of [C | T2true] ----
        nc.scalar.activation(rcol[:], scol[:], Act.Exp, scale=0.8)
        vp_blocks = [b for b in range(NB) if mask_eng[b] != "a"]
        a_blocks = [b for b in range(NB) if mask_eng[b] == "a"]
        bd_ps = sps.tile([1, RC], f32, tag="pj", name="bd_ps")
        for i, b in enumerate(vp_blocks):
            nc.tensor.matmul(bd_ps[:], ones_col_bf[:], r_v[:, b, 0:RC],
                             start=(i == 0), stop=(i == len(vp_blocks) - 1))
        if a_blocks:
            ba_ps = sps.tile([1, RC], f32, tag="pj", name="ba_ps")
            for i, b in enumerate(a_blocks):
                nc.tensor.matmul(ba_ps[:], ones_col_bf[:], r_v[:, b, 0:RC],
                                 start=(i == 0), stop=(i == len(a_blocks) - 1))
            # C -> bf16 hi/lo rows for the per-bank K=2 correction matmul
            # (row 1 is written via a tiny SBUF->SBUF DMA: engines cannot
            # address a tile starting at partition 1)
            nc.vector.tensor_copy(chl[0:1, :], ba_ps[:])
            nc.vector.tensor_tensor(clo_f[:], ba_ps[:], chl[0:1, :],
                                    op=Alu.subtract)
            nc.vector.tensor_copy(clo_bf[:], clo_f[:])
            nc.sync.dma_start(chl[1:2, :], clo_bf[:])
            bdt2 = persist.tile([1, 65], f32)
            nc.vector.tensor_copy(bdt2[:], bd_ps[:, F + 1:RC])
            nc.vector.scalar_tensor_tensor(cacc[:], ba_ps[:, F + 1:RC],
                                           2.0, bdt2[:],
                                           op0=Alu.mult, op1=Alu.add)
        else:
            nc.vector.tensor_copy(cacc[:], bd_ps[:, F + 1:RC])
        crep_ps = sps.tile([P, 65], f32, tag="pj", name="crep_ps")
        nc.tensor.matmul(crep_ps[:], ones_row_f[:], cacc[:])
        nc.scalar.copy(crep[:], crep_ps[:])

        # Striped block order for mask emission + chain consumption: each
        # sup's emission quota mixes DVE and ACT blocks proportionally so
        # both engines generate masks concurrently instead of in phases.
        vblocks = [b for b in range(NB) if mask_eng[b] == "v"]
        pblocks = [b for b in range(NB) if mask_eng[b] == "p"]
        ablocks = [b for b in range(NB) if mask_eng[b] == "a"]
        ORDER = []
        for h in range(SB):
            for cls in (vblocks, pblocks, ablocks):
                n0 = len(cls) * h // SB
                n1 = len(cls) * (h + 1) // SB
                ORDER.extend(cls[n0:n1])
        assert sorted(ORDER) == list(range(NB))

        # ---- epilogue ----
        late_pend = []

        def late_phase(lsup):
            """num/den algebra + out DMA for i-blocks 2*lsup, 2*lsup+1.
            eps[0:65] is already r-scaled by the drain; everything here is
            a plain Pool tensor_tensor (ACT/DVE stay free for masks)."""
            i0 = ISUP * lsup
            dd = ep.tile([P, ISUP], f32, tag="dd", name="dd")
            nc.gpsimd.tensor_tensor(dd[:],
                                    crep[:, 64:65].to_broadcast([P, ISUP]),
                                    eps_v[:, i0:i0 + ISUP, 129], op=Alu.subtract)
            den = ep.tile([P, ISUP], f32, tag="den", name="den")
            nc.gpsimd.tensor_tensor(den[:], eps_v[:, i0:i0 + ISUP, F], dd[:],
                                    op=Alu.add)
            rden = ep.tile([P, ISUP], f32, tag="rden", name="rden")
            nc.vector.reciprocal(rden[:], den[:])
            ost = osp.tile([P, ISUP * F], f32, tag="ost", name="ost")
            for t in range(ISUP):
                i = i0 + t
                e0 = RC * i
                B = ep.tile([P, F], f32, tag="nB", name="nB")
                nc.gpsimd.tensor_tensor(B[:], crep[:, 0:F],
                                        eps_all[:, e0 + F + 1:e0 + 2 * F + 1],
                                        op=Alu.subtract)
                num = ep.tile([P, F], f32, tag="num", name="num")
                nc.gpsimd.tensor_tensor(num[:], eps_all[:, e0:e0 + F], B[:],
                                        op=Alu.add)
                nc.gpsimd.tensor_tensor(ost[:, F * t:F * (t + 1)], num[:],
                                        rden[:, t:t + 1].to_broadcast([P, F]),
                                        op=Alu.mult)
            nc.sync.dma_start(
                out_d.rearrange("(c p) f -> p c f", p=P)[:, ISUP * lsup:ISUP * (lsup + 1), :],
                ost.rearrange("p (c f) -> p c f", f=F))

        # ---- main loop over sups (sup 0's matmuls were emitted in setup) ----
        for sup in range(NSUP):
            batch = sup // SB
            nb_ = batch + 1
            if nb_ < NBATCH:
                per = NB // SB
                half = sup % SB
                for b in ORDER[per * half:per * (half + 1)]:
                    emit_mask(nb_, b)
            acc = get_acc(sup)
            if sup > 0:
                off = (sup % SB) * ISUP * P
                for bi, b in enumerate(ORDER):
                    mt = mask_tiles[(batch, b)]
                    for t in range(ISUP):
                        nc.tensor.matmul(acc[:, 512 * t:512 * t + RC],
                                         mt[:, off + t * P:off + (t + 1) * P],
                                         r_v[:, b, 0:RC],
                                         start=(bi == 0),
                                         stop=(bi == NB - 1 and GA == 0))
            if GA > 0:
                # add the +-1-block column correction C into each bank
                for t in range(ISUP):
                    nc.tensor.matmul(acc[:, 512 * t:512 * t + RC],
                                     ones2[:], chl[:],
                                     start=False, stop=True)
            if sup % SB == SB - 1:
                for b in range(NB):
                    mask_tiles.pop((batch, b))
            for t in range(ISUP):
                i = sup * ISUP + t
                # r-scale R1|v during the drain; plain-copy R2|v2
                nc.vector.tensor_scalar(eps_all[:, RC * i:RC * i + F + 1],
                                        acc[:, 512 * t:512 * t + F + 1],
                                        rcol[:, i:i + 1], None, op0=Alu.mult)
                nc.vector.tensor_copy(eps_all[:, RC * i + F + 1:RC * (i + 1)],
                                      acc[:, 512 * t + F + 1:512 * t + RC])
            late_pend.append(sup)
            if len(late_pend) > 1:
                late_phase(late_pend.pop(0))
        for pr in late_pend:
            late_phase(pr)

    nc.compile()
    return nc


def _get_nc():
    if "nc" not in _CACHE:
        _CACHE["nc"] = _build()
    return _CACHE["nc"]


def kernel(graph, W, a):
    from concourse.bass_utils import run_bass_kernel_spmd

    graph = np.asarray(graph, dtype=np.float32)
    W = np.asarray(W, dtype=np.float32)
    a = np.asarray(a, dtype=np.float32)
    gt = np.ascontiguousarray(graph.T)

    nc = _get_nc()
    in_maps = [
        {
            "gt": gt,
            "w": np.ascontiguousarray(W[h]),
            "wt": np.ascontiguousarray(W[h].T),
            "a2": np.ascontiguousarray(a[h].reshape(2, F).T),
        }
        for h in range(H)
    ]
    trace = bool(int(os.environ.get("GAT_TRACE", "0")))
    res = run_bass_kernel_spmd(nc, in_maps, core_ids=list(range(H)), trace=trace)
    _CACHE["last_result"] = res
    return np.stack([res.results[h]["out"] for h in range(H)], axis=0)


# revision 61
# speedup vs baseline: 1.1892x; 1.0114x over previous
"""Multi-head GAT layer for Trainium2 — 8 heads sharded across 8 NeuronCores.

Per head h (N=4096 nodes, F=64 features):
    ltg   = graph @ W[h]                          [N, F]
    s     = ltg @ a_src,  d = ltg @ a_dst         [N]
    E     = leaky_relu(s[:, None] + d[None, :], 0.2)
    Alpha = softmax(E, axis=-1)
    out   = Alpha @ ltg

Algebraic core (as v1): with z = s_i + d_j, M_ij = [z >= 0],
    exp(leaky_relu(z)) = M e^{s_i} e^{d_j} + (1-M) e^{0.2 s_i} e^{0.2 d_j}
so softmax reduces to mask generation (DVE/ACT/Pool) + masked matmuls
(PE) over R = [R1|R2|v|v2] = [e^d.ltg | e^{.2d}.ltg | e^d | e^{.2d}],
with num_i/e^{.2 s_i} = r_i (M@R1)_i + (T2 - M@R2)_i, r = e^{.8 s}.

v2 restructure (vs the 141.6us v1):
  * host passes graph^T / W^T / a^T (pure layout) so the graph loads as
    4 large DMAs and the 32 PE transposes + their DVE/ACT drains vanish.
  * ACT-generated masks use Sign (+-1, in every ACT table set) instead
    of saturated Sigmoid: zero ACT_TABLE_LOAD thrash against the exps.
    +-1 blocks store R/2 (exp bias -ln2); the constant column error
    C = sum_ACT T_b/2 is added back during the psum->SBUF drain via a
    broadcast C tensor, so the epilogue algebra is unchanged.
  * PSUM: 2-bank accumulators (2 i-blocks/sup, 16 sups) double-buffered
    -> no per-sup drain stall; setup keeps 4 banks for projections and
    the s-broadcast.
  * masks batched [128, 512] (2 sups per instruction) to amortize the
    fixed per-instruction cost on DVE (~105ns) and ACT (~293ns).
  * out rows staged [128, 256] and DMA'd per 4 blocks (8 descriptors
    instead of 32 dma_starts).

Heads are fully independent: core h computes head h; no collectives.
"""

import math
import os
from contextlib import ExitStack

import numpy as np

N, F_IN, F, H = 4096, 64, 64, 8
P = 128
NB = N // P           # 32 node blocks (j)
NG = NB // 4          # 8 groups of 4 blocks
ISUP = 2              # i-blocks per PSUM accumulator (2 banks)
NSUP = NB // ISUP     # 16 sup iterations
SB = int(os.environ.get("GAT_SB", "2"))  # sups per mask batch
NBATCH = NSUP // SB   # mask batches
RC = 130              # R columns per j-block: R1(64) | v | R2(64) | v2
                      # (r-scaled part contiguous at 0:65 for the drain)
LN2 = math.log(2.0)

# mask-engine split in GROUPS of 4 blocks: (DVE, Pool, ACT).
# Pool is_ge is software-emulated at ~8.7us per [128,512] tile and starves
# DVE via SBUF port contention -> keep GP=0.
GV = int(os.environ.get("GAT_GV", "5"))
GP = int(os.environ.get("GAT_GP", "0"))
GA = int(os.environ.get("GAT_GA", "3"))
assert GV + GP + GA == NG
_CACHE = {}


def _build():
    import concourse.bass as bass  # noqa: F401
    import concourse.mybir as mybir
    import concourse.tile as tile
    from concourse import bacc

    dt = mybir.dt
    f32 = dt.float32
    f16 = dt.float16
    bf16 = dt.bfloat16
    Alu = mybir.AluOpType
    Act = mybir.ActivationFunctionType

    mask_eng = (["v"] * (4 * GV) + ["p"] * (4 * GP) + ["a"] * (4 * GA))

    nc = bacc.Bacc("TRN2", debug=False, num_devices=H)
    gt_d = nc.dram_tensor("gt", [F_IN, N], f32, kind="ExternalInput").ap()
    w_d = nc.dram_tensor("w", [F_IN, F], f32, kind="ExternalInput").ap()
    wt_d = nc.dram_tensor("wt", [F, F_IN], f32, kind="ExternalInput").ap()
    a2_d = nc.dram_tensor("a2", [F, 2], f32, kind="ExternalInput").ap()
    out_d = nc.dram_tensor("out", [N, F], f32, kind="ExternalOutput").ap()

    with tile.TileContext(nc) as tc, ExitStack() as ctx:
        persist = ctx.enter_context(tc.tile_pool(name="persist", bufs=1))
        sps = ctx.enter_context(tc.tile_pool(name="sps", bufs=3, space="PSUM"))
        bcp = ctx.enter_context(tc.tile_pool(name="bcp", bufs=1, space="PSUM"))
        accp = ctx.enter_context(tc.tile_pool(name="acc", bufs=2, space="PSUM"))
        mp = ctx.enter_context(tc.tile_pool(name="mask", bufs=2))
        ep = ctx.enter_context(tc.tile_pool(name="ep", bufs=6))
        osp = ctx.enter_context(tc.tile_pool(name="osp", bufs=2))

        # ---- input DMAs ----
        # Group processing order: chunk 0 first (batch-0 masks need
        # s_rep[:, 0:512]), then the ACT-mask groups so ACT's Sign masks
        # start early, then the remaining DVE groups.
        GORDER = [0] + list(range(GV + GP, NG)) + [g for g in range(1, GV + GP)]
        gsp = ctx.enter_context(tc.tile_pool(name="gsp", bufs=4))
        gviews = {}
        gst = gsp.tile([F_IN, 512], f32, tag="g0", name="gst0", bufs=1)
        nc.sync.dma_start(gst[:], gt_d[:, 512 * GORDER[0]:512 * (GORDER[0] + 1)])
        gviews[GORDER[0]] = gst
        wf = persist.tile([F_IN, F], f32)
        nc.sync.dma_start(wf[:], w_d[:])
        wt_sb = persist.tile([F, F_IN], f32)
        nc.sync.dma_start(wt_sb[:], wt_d[:])
        a2_sb = persist.tile([F, 2], f32)
        nc.sync.dma_start(a2_sb[:], a2_d[:])
        for c in GORDER[1:]:
            gst = gsp.tile([F_IN, 512], f32, tag="gst", name="gst")
            nc.sync.dma_start(gst[:], gt_d[:, 512 * c:512 * (c + 1)])
            gviews[c] = gst

        ones_row_bf = persist.tile([1, P], bf16)
        nc.vector.memset(ones_row_bf[:], 1.0)
        ones_col_bf = persist.tile([P, 1], bf16)
        nc.gpsimd.memset(ones_col_bf[:], 1.0)
        ones_row_f = persist.tile([1, P], f32)
        nc.vector.memset(ones_row_f[:], 1.0)
        negln2 = persist.tile([P, 1], f32)
        nc.vector.memset(negln2[:], -LN2)

        # fused fp16 [W | w_s | w_d]; w_sd = W @ a2 via host-provided W^T
        w16 = persist.tile([F_IN, F + 2], f16)
        nc.gpsimd.tensor_copy(w16[:, 0:F], wf[:])
        wsd_ps = sps.tile([F_IN, 2], f32, tag="pj", name="wsd_ps")
        nc.tensor.matmul(wsd_ps[:], wt_sb[:], a2_sb[:])
        nc.scalar.copy(w16[:, F:F + 2], wsd_ps[:])

        gT16 = persist.tile([F_IN, N], f16)
        sdrow = persist.tile([2, N], bf16)
        s_rep = persist.tile([P, N], bf16)
        r_all = persist.tile([P, RC * NB], bf16)
        r_v = r_all.rearrange("p (b c) -> p b c", c=RC)
        eps_all = persist.tile([P, RC * NB], f32)
        eps_v = eps_all.rearrange("p (b c) -> p b c", c=RC)
        scol = persist.tile([P, NB], f32)
        dcol = persist.tile([P, NB], f32)
        negd = persist.tile([P, NB], f32)
        vcol = persist.tile([P, NB], f32)
        v2col = persist.tile([P, NB], f32)
        rcol = persist.tile([P, NB], f32)
        crep = persist.tile([P, 65], f32)        # [R2 totals | v2 total] bcast
        cacc = persist.tile([1, 65], f32)
        chl = persist.tile([2, RC], bf16)        # C as bf16 hi/lo rows
        clo_f = persist.tile([1, RC], f32)
        clo_bf = persist.tile([1, RC], bf16)
        ones2 = persist.tile([2, P], bf16)
        nc.vector.memset(ones2[:], 1.0)

        mask_tiles = {}
        acc_tiles = {}

        def get_acc(sup):
            if sup not in acc_tiles:
                acc_tiles[sup] = accp.tile([P, 1024], f32, tag="acc",
                                           name=f"acc{sup}")
            return acc_tiles[sup]

        def emit_mask(batch, b):
            eng = mask_eng[b]
            i0 = batch * SB * ISUP * P
            mt = mp.tile([P, SB * ISUP * P], bf16, tag=f"m{b}", name=f"mask{b}")
            src = s_rep[:, i0:i0 + SB * ISUP * P]
            if eng == "a":
                nc.scalar.activation(mt[:], src, Act.Sign, bias=dcol[:, b:b + 1])
            elif eng == "v":
                nc.vector.tensor_scalar(mt[:], src, negd[:, b:b + 1], None,
                                        op0=Alu.is_ge)
            else:
                nc.gpsimd.tensor_scalar(mt[:], src, negd[:, b:b + 1], None,
                                        op0=Alu.is_ge)
            mask_tiles[(batch, b)] = mt

        def any_copy(eng, dst, src):
            if eng is nc.scalar:
                eng.copy(dst, src)
            else:
                eng.tensor_copy(dst, src)

        # ---- graph cast + s-row broadcast, per 512-col chunk ----
        cast_eng = [nc.vector, nc.gpsimd, nc.gpsimd, nc.gpsimd,
                    nc.scalar, nc.gpsimd, nc.gpsimd, nc.gpsimd]
        # ---- per-chunk fused setup: cast + s-broadcast + projection group.
        # Group g needs exactly gT16 chunk g, and allocating the pj-ring
        # tiles in time order keeps the ring from gating early groups on
        # late DMA chunks.
        chain0 = [4 * g + k for g in GORDER for k in range(4)]
        for gi, g in enumerate(GORDER):
            any_copy(cast_eng[gi], gT16[:, 512 * g:512 * (g + 1)],
                     gviews[g][:])
            srow_ps = sps.tile([2, 512], f32, tag="pj", name="srow_ps")
            nc.tensor.matmul(srow_ps[:], w16[:, F:F + 2],
                             gT16[:, 512 * g:512 * (g + 1)])
            nc.vector.tensor_copy(sdrow[:, 512 * g:512 * (g + 1)],
                                  srow_ps[:])
            bc_ps = bcp.tile([P, 512], f32, tag="bc", name="bc_ps")
            nc.tensor.matmul(bc_ps[:],
                             ones_row_bf[:], sdrow[0:1, 512 * g:512 * (g + 1)])
            seng = nc.vector if g % 2 == 0 else nc.scalar
            any_copy(seng, s_rep[:, 512 * g:512 * (g + 1)], bc_ps[:])

            # projection + R construction for this group's 4 blocks
            is_a = mask_eng[4 * g] == "a"
            prj = sps.tile([P, 4 * 66], f32, tag="pj", name="prj")
            prj_v = prj.rearrange("p (b c) -> p b c", c=66)
            for k in range(4):
                b = 4 * g + k
                nc.tensor.matmul(prj[:, 66 * k:66 * (k + 1)],
                                 gT16[:, P * b:P * (b + 1)], w16[:])
            bsl = slice(4 * g, 4 * g + 4)
            d_src = prj_v[:, :, 65]
            s_src = prj_v[:, :, 64]
            nc.vector.tensor_copy(scol[:, bsl], s_src)
            nc.vector.tensor_copy(dcol[:, bsl], d_src)
            nc.vector.tensor_scalar(negd[:, bsl], d_src, -1.0, None,
                                    op0=Alu.mult)
            # v = e^d, v2 = e^{0.2 d} (x1/2 for +-1 blocks)
            bias = negln2[:] if is_a else 0.0
            nc.scalar.activation(vcol[:, bsl], dcol[:, bsl], Act.Exp,
                                 bias=bias)
            nc.scalar.activation(v2col[:, bsl], dcol[:, bsl], Act.Exp,
                                 scale=0.2, bias=bias)
            for k in range(4):
                b = 4 * g + k
                r0 = RC * b
                nc.vector.tensor_scalar(r_all[:, r0:r0 + F], prj_v[:, k, 0:F],
                                        vcol[:, b:b + 1], None, op0=Alu.mult)
                nc.vector.tensor_tensor(
                    r_all[:, r0 + F + 1:r0 + 2 * F + 1],
                    prj_v[:, k, 0:F],
                    v2col[:, b:b + 1].to_broadcast([P, F]),
                    op=Alu.mult)
            nc.gpsimd.tensor_copy(r_v[:, bsl, F], vcol[:, bsl])
            nc.gpsimd.tensor_copy(r_v[:, bsl, 129], v2col[:, bsl])
            # batch-0 masks + sup-0 accumulation for this group's blocks
            acc0 = get_acc(0)
            for k in range(4):
                b = 4 * g + k
                emit_mask(0, b)
                for t in range(ISUP):
                    nc.tensor.matmul(acc0[:, 512 * t:512 * t + RC],
                                     mask_tiles[(0, b)][:, t * P:(t + 1) * P],
                                     r_v[:, b, 0:RC],
                                     start=(b == chain0[0]),
                                     stop=(b == chain0[-1] and GA == 0))

        # ---- rcol + column-total bursts + broadcast of [C | T2true] ----
        nc.scalar.activation(rcol[:], scol[:], Act.Exp, scale=0.8)
        vp_blocks = [b for b in range(NB) if mask_eng[b] != "a"]
        a_blocks = [b for b in range(NB) if mask_eng[b] == "a"]
        bd_ps = sps.tile([1, RC], f32, tag="pj", name="bd_ps")
        for i, b in enumerate(vp_blocks):
            nc.tensor.matmul(bd_ps[:], ones_col_bf[:], r_v[:, b, 0:RC],
                             start=(i == 0), stop=(i == len(vp_blocks) - 1))
        if a_blocks:
            ba_ps = sps.tile([1, RC], f32, tag="pj", name="ba_ps")
            for i, b in enumerate(a_blocks):
                nc.tensor.matmul(ba_ps[:], ones_col_bf[:], r_v[:, b, 0:RC],
                                 start=(i == 0), stop=(i == len(a_blocks) - 1))
            # C -> bf16 hi/lo rows for the per-bank K=2 correction matmul
            # (row 1 is written via a tiny SBUF->SBUF DMA: engines cannot
            # address a tile starting at partition 1)
            nc.vector.tensor_copy(chl[0:1, :], ba_ps[:])
            nc.vector.tensor_tensor(clo_f[:], ba_ps[:], chl[0:1, :],
                                    op=Alu.subtract)
            nc.vector.tensor_copy(clo_bf[:], clo_f[:])
            nc.sync.dma_start(chl[1:2, :], clo_bf[:])
            bdt2 = persist.tile([1, 65], f32)
            nc.vector.tensor_copy(bdt2[:], bd_ps[:, F + 1:RC])
            nc.vector.scalar_tensor_tensor(cacc[:], ba_ps[:, F + 1:RC],
                                           2.0, bdt2[:],
                                           op0=Alu.mult, op1=Alu.add)
        else:
            nc.vector.tensor_copy(cacc[:], bd_ps[:, F + 1:RC])
        crep_ps = sps.tile([P, 65], f32, tag="pj", name="crep_ps")
        nc.tensor.matmul(crep_ps[:], ones_row_f[:], cacc[:])
        nc.scalar.copy(crep[:], crep_ps[:])

        # Striped block order for mask emission + chain consumption: each
        # sup's emission quota mixes DVE and ACT blocks proportionally so
        # both engines generate masks concurrently instead of in phases.
        vblocks = [b for b in range(NB) if mask_eng[b] == "v"]
        pblocks = [b for b in range(NB) if mask_eng[b] == "p"]
        ablocks = [b for b in range(NB) if mask_eng[b] == "a"]
        ORDER = []
        for h in range(SB):
            for cls in (vblocks, pblocks, ablocks):
                n0 = len(cls) * h // SB
                n1 = len(cls) * (h + 1) // SB
                ORDER.extend(cls[n0:n1])
        assert sorted(ORDER) == list(range(NB))

        # ---- epilogue ----
        late_pend = []

        def late_phase(lsup):
            """num/den algebra + out DMA for i-blocks 2*lsup, 2*lsup+1.
            eps[0:65] is already r-scaled by the drain; everything here is
            a plain Pool tensor_tensor (ACT/DVE stay free for masks)."""
            i0 = ISUP * lsup
            # the final sups' epilogue rides the by-then-idle DVE instead
            # of waiting out Pool's queue
            te = nc.vector if lsup >= NSUP - 2 else nc.gpsimd
            dd = ep.tile([P, ISUP], f32, tag="dd", name="dd")
            te.tensor_tensor(dd[:],
                             crep[:, 64:65].to_broadcast([P, ISUP]),
                             eps_v[:, i0:i0 + ISUP, 129], op=Alu.subtract)
            den = ep.tile([P, ISUP], f32, tag="den", name="den")
            te.tensor_tensor(den[:], eps_v[:, i0:i0 + ISUP, F], dd[:],
                             op=Alu.add)
            rden = ep.tile([P, ISUP], f32, tag="rden", name="rden")
            nc.vector.reciprocal(rden[:], den[:])
            ost = osp.tile([P, ISUP * F], f32, tag="ost", name="ost")
            for t in range(ISUP):
                i = i0 + t
                e0 = RC * i
                B = ep.tile([P, F], f32, tag="nB", name="nB")
                te.tensor_tensor(B[:], crep[:, 0:F],
                                 eps_all[:, e0 + F + 1:e0 + 2 * F + 1],
                                 op=Alu.subtract)
                num = ep.tile([P, F], f32, tag="num", name="num")
                te.tensor_tensor(num[:], eps_all[:, e0:e0 + F], B[:],
                                 op=Alu.add)
                te.tensor_tensor(ost[:, F * t:F * (t + 1)], num[:],
                                 rden[:, t:t + 1].to_broadcast([P, F]),
                                 op=Alu.mult)
            nc.sync.dma_start(
                out_d.rearrange("(c p) f -> p c f", p=P)[:, ISUP * lsup:ISUP * (lsup + 1), :],
                ost.rearrange("p (c f) -> p c f", f=F))

        # ---- main loop over sups (sup 0's matmuls were emitted in setup) ----
        for sup in range(NSUP):
            batch = sup // SB
            nb_ = batch + 1
            if nb_ < NBATCH:
                per = NB // SB
                half = sup % SB
                for b in ORDER[per * half:per * (half + 1)]:
                    emit_mask(nb_, b)
            acc = get_acc(sup)
            if sup > 0:
                off = (sup % SB) * ISUP * P
                for bi, b in enumerate(ORDER):
                    mt = mask_tiles[(batch, b)]
                    for t in range(ISUP):
                        nc.tensor.matmul(acc[:, 512 * t:512 * t + RC],
                                         mt[:, off + t * P:off + (t + 1) * P],
                                         r_v[:, b, 0:RC],
                                         start=(bi == 0),
                                         stop=(bi == NB - 1 and GA == 0))
            if GA > 0:
                # add the +-1-block column correction C into each bank
                for t in range(ISUP):
                    nc.tensor.matmul(acc[:, 512 * t:512 * t + RC],
                                     ones2[:], chl[:],
                                     start=False, stop=True)
            if sup % SB == SB - 1:
                for b in range(NB):
                    mask_tiles.pop((batch, b))
            for t in range(ISUP):
                i = sup * ISUP + t
                # r-scale R1|v during the drain; plain-copy R2|v2
                nc.vector.tensor_scalar(eps_all[:, RC * i:RC * i + F + 1],
                                        acc[:, 512 * t:512 * t + F + 1],
                                        rcol[:, i:i + 1], None, op0=Alu.mult)
                nc.vector.tensor_copy(eps_all[:, RC * i + F + 1:RC * (i + 1)],
                                      acc[:, 512 * t + F + 1:512 * t + RC])
            late_pend.append(sup)
            if len(late_pend) > 1:
                late_phase(late_pend.pop(0))
        for pr in late_pend:
            late_phase(pr)

    nc.compile()
    return nc


def _get_nc():
    if "nc" not in _CACHE:
        _CACHE["nc"] = _build()
    return _CACHE["nc"]


def kernel(graph, W, a):
    from concourse.bass_utils import run_bass_kernel_spmd

    graph = np.asarray(graph, dtype=np.float32)
    W = np.asarray(W, dtype=np.float32)
    a = np.asarray(a, dtype=np.float32)
    gt = np.ascontiguousarray(graph.T)

    nc = _get_nc()
    in_maps = [
        {
            "gt": gt,
            "w": np.ascontiguousarray(W[h]),
            "wt": np.ascontiguousarray(W[h].T),
            "a2": np.ascontiguousarray(a[h].reshape(2, F).T),
        }
        for h in range(H)
    ]
    trace = bool(int(os.environ.get("GAT_TRACE", "0")))
    res = run_bass_kernel_spmd(nc, in_maps, core_ids=list(range(H)), trace=trace)
    _CACHE["last_result"] = res
    return np.stack([res.results[h]["out"] for h in range(H)], axis=0)


# revision 62
# speedup vs baseline: 1.2073x; 1.0152x over previous
"""Multi-head GAT layer for Trainium2 — 8 heads sharded across 8 NeuronCores.

Per head h (N=4096 nodes, F=64 features):
    ltg   = graph @ W[h]                          [N, F]
    s     = ltg @ a_src,  d = ltg @ a_dst         [N]
    E     = leaky_relu(s[:, None] + d[None, :], 0.2)
    Alpha = softmax(E, axis=-1)
    out   = Alpha @ ltg

Algebraic core (as v1): with z = s_i + d_j, M_ij = [z >= 0],
    exp(leaky_relu(z)) = M e^{s_i} e^{d_j} + (1-M) e^{0.2 s_i} e^{0.2 d_j}
so softmax reduces to mask generation (DVE/ACT/Pool) + masked matmuls
(PE) over R = [R1|R2|v|v2] = [e^d.ltg | e^{.2d}.ltg | e^d | e^{.2d}],
with num_i/e^{.2 s_i} = r_i (M@R1)_i + (T2 - M@R2)_i, r = e^{.8 s}.

v2 restructure (vs the 141.6us v1):
  * host passes graph^T / W^T / a^T (pure layout) so the graph loads as
    4 large DMAs and the 32 PE transposes + their DVE/ACT drains vanish.
  * ACT-generated masks use Sign (+-1, in every ACT table set) instead
    of saturated Sigmoid: zero ACT_TABLE_LOAD thrash against the exps.
    +-1 blocks store R/2 (exp bias -ln2); the constant column error
    C = sum_ACT T_b/2 is added back during the psum->SBUF drain via a
    broadcast C tensor, so the epilogue algebra is unchanged.
  * PSUM: 2-bank accumulators (2 i-blocks/sup, 16 sups) double-buffered
    -> no per-sup drain stall; setup keeps 4 banks for projections and
    the s-broadcast.
  * masks batched [128, 512] (2 sups per instruction) to amortize the
    fixed per-instruction cost on DVE (~105ns) and ACT (~293ns).
  * out rows staged [128, 256] and DMA'd per 4 blocks (8 descriptors
    instead of 32 dma_starts).

Heads are fully independent: core h computes head h; no collectives.
"""

import math
import os
from contextlib import ExitStack

import numpy as np

N, F_IN, F, H = 4096, 64, 64, 8
P = 128
NB = N // P           # 32 node blocks (j)
NG = NB // 4          # 8 groups of 4 blocks
ISUP = 2              # i-blocks per PSUM accumulator (2 banks)
NSUP = NB // ISUP     # 16 sup iterations
SB = int(os.environ.get("GAT_SB", "2"))  # sups per mask batch
NBATCH = NSUP // SB   # mask batches
RC = 130              # R columns per j-block: R1(64) | v | R2(64) | v2
                      # (r-scaled part contiguous at 0:65 for the drain)
LN2 = math.log(2.0)

# mask-engine split in GROUPS of 4 blocks: (DVE, Pool, ACT).
# Pool is_ge is software-emulated at ~8.7us per [128,512] tile and starves
# DVE via SBUF port contention -> keep GP=0.
GV = int(os.environ.get("GAT_GV", "5"))
GP = int(os.environ.get("GAT_GP", "0"))
GA = int(os.environ.get("GAT_GA", "3"))
assert GV + GP + GA == NG
_CACHE = {}


def _build():
    import concourse.bass as bass  # noqa: F401
    import concourse.mybir as mybir
    import concourse.tile as tile
    from concourse import bacc

    dt = mybir.dt
    f32 = dt.float32
    f16 = dt.float16
    bf16 = dt.bfloat16
    Alu = mybir.AluOpType
    Act = mybir.ActivationFunctionType

    mask_eng = (["v"] * (4 * GV) + ["p"] * (4 * GP) + ["a"] * (4 * GA))

    nc = bacc.Bacc("TRN2", debug=False, num_devices=H)
    gt_d = nc.dram_tensor("gt", [F_IN, N], f32, kind="ExternalInput").ap()
    w_d = nc.dram_tensor("w", [F_IN, F], f32, kind="ExternalInput").ap()
    wt_d = nc.dram_tensor("wt", [F, F_IN], f32, kind="ExternalInput").ap()
    a2_d = nc.dram_tensor("a2", [F, 2], f32, kind="ExternalInput").ap()
    out_d = nc.dram_tensor("out", [N, F], f32, kind="ExternalOutput").ap()

    with tile.TileContext(nc) as tc, ExitStack() as ctx:
        persist = ctx.enter_context(tc.tile_pool(name="persist", bufs=1))
        sps = ctx.enter_context(tc.tile_pool(name="sps", bufs=3, space="PSUM"))
        bcp = ctx.enter_context(tc.tile_pool(name="bcp", bufs=1, space="PSUM"))
        accp = ctx.enter_context(tc.tile_pool(name="acc", bufs=2, space="PSUM"))
        mp = ctx.enter_context(tc.tile_pool(name="mask", bufs=2))
        ep = ctx.enter_context(tc.tile_pool(name="ep", bufs=6))
        osp = ctx.enter_context(tc.tile_pool(name="osp", bufs=2))

        # ---- input DMAs ----
        # Group processing order: chunk 0 first (batch-0 masks need
        # s_rep[:, 0:512]), then the ACT-mask groups so ACT's Sign masks
        # start early, then the remaining DVE groups.
        GORDER = [0] + list(range(GV + GP, NG)) + [g for g in range(1, GV + GP)]
        gsp = ctx.enter_context(tc.tile_pool(name="gsp", bufs=4))
        gviews = {}
        gst = gsp.tile([F_IN, 512], f32, tag="g0", name="gst0", bufs=1)
        nc.sync.dma_start(gst[:], gt_d[:, 512 * GORDER[0]:512 * (GORDER[0] + 1)])
        gviews[GORDER[0]] = gst
        wf = persist.tile([F_IN, F], f32)
        nc.sync.dma_start(wf[:], w_d[:])
        wt_sb = persist.tile([F, F_IN], f32)
        nc.sync.dma_start(wt_sb[:], wt_d[:])
        a2_sb = persist.tile([F, 2], f32)
        nc.sync.dma_start(a2_sb[:], a2_d[:])
        for c in GORDER[1:]:
            gst = gsp.tile([F_IN, 512], f32, tag="gst", name="gst")
            nc.sync.dma_start(gst[:], gt_d[:, 512 * c:512 * (c + 1)])
            gviews[c] = gst

        ones_row_bf = persist.tile([1, P], bf16)
        nc.vector.memset(ones_row_bf[:], 1.0)
        ones_col_bf = persist.tile([P, 1], bf16)
        nc.gpsimd.memset(ones_col_bf[:], 1.0)
        ones_row_f = persist.tile([1, P], f32)
        nc.vector.memset(ones_row_f[:], 1.0)
        negln2 = persist.tile([P, 1], f32)
        nc.vector.memset(negln2[:], -LN2)

        # fused fp16 [W | w_s | w_d]; w_sd = W @ a2 via host-provided W^T
        w16 = persist.tile([F_IN, F + 2], f16)
        nc.gpsimd.tensor_copy(w16[:, 0:F], wf[:])
        wsd_ps = sps.tile([F_IN, 2], f32, tag="pj", name="wsd_ps")
        nc.tensor.matmul(wsd_ps[:], wt_sb[:], a2_sb[:])
        nc.scalar.copy(w16[:, F:F + 2], wsd_ps[:])

        gT16 = persist.tile([F_IN, N], f16)
        sdrow = persist.tile([2, N], bf16)
        s_rep = persist.tile([P, N], bf16)
        r_all = persist.tile([P, RC * NB], bf16)
        r_v = r_all.rearrange("p (b c) -> p b c", c=RC)
        eps_all = persist.tile([P, RC * NB], f32)
        eps_v = eps_all.rearrange("p (b c) -> p b c", c=RC)
        scol = persist.tile([P, NB], f32)
        dcol = persist.tile([P, NB], f32)
        negd = persist.tile([P, NB], f32)
        vcol = persist.tile([P, NB], f32)
        v2col = persist.tile([P, NB], f32)
        rcol = persist.tile([P, NB], f32)
        crep = persist.tile([P, 65], f32)        # [R2 totals | v2 total] bcast
        cacc = persist.tile([1, 65], f32)
        chl = persist.tile([2, RC], bf16)        # C as bf16 hi/lo rows
        clo_f = persist.tile([1, RC], f32)
        clo_bf = persist.tile([1, RC], bf16)
        ones2 = persist.tile([2, P], bf16)
        nc.vector.memset(ones2[:], 1.0)

        mask_tiles = {}
        acc_tiles = {}

        def get_acc(sup):
            if sup not in acc_tiles:
                acc_tiles[sup] = accp.tile([P, 1024], f32, tag="acc",
                                           name=f"acc{sup}")
            return acc_tiles[sup]

        def emit_mask(batch, b):
            eng = mask_eng[b]
            i0 = batch * SB * ISUP * P
            mt = mp.tile([P, SB * ISUP * P], bf16, tag=f"m{b}", name=f"mask{b}")
            src = s_rep[:, i0:i0 + SB * ISUP * P]
            if eng == "a":
                nc.scalar.activation(mt[:], src, Act.Sign, bias=dcol[:, b:b + 1])
            elif eng == "v":
                nc.vector.tensor_scalar(mt[:], src, negd[:, b:b + 1], None,
                                        op0=Alu.is_ge)
            else:
                nc.gpsimd.tensor_scalar(mt[:], src, negd[:, b:b + 1], None,
                                        op0=Alu.is_ge)
            mask_tiles[(batch, b)] = mt

        def any_copy(eng, dst, src):
            if eng is nc.scalar:
                eng.copy(dst, src)
            else:
                eng.tensor_copy(dst, src)

        # ---- graph cast + s-row broadcast, per 512-col chunk ----
        cast_eng = [nc.vector, nc.gpsimd, nc.gpsimd, nc.gpsimd,
                    nc.scalar, nc.gpsimd, nc.gpsimd, nc.gpsimd]
        # ---- per-chunk fused setup: cast + s-broadcast + projection group.
        # Group g needs exactly gT16 chunk g, and allocating the pj-ring
        # tiles in time order keeps the ring from gating early groups on
        # late DMA chunks.
        chain0 = [4 * g + k for g in GORDER for k in range(4)]
        for gi, g in enumerate(GORDER):
            any_copy(cast_eng[gi], gT16[:, 512 * g:512 * (g + 1)],
                     gviews[g][:])
            srow_ps = sps.tile([2, 512], f32, tag="pj", name="srow_ps")
            nc.tensor.matmul(srow_ps[:], w16[:, F:F + 2],
                             gT16[:, 512 * g:512 * (g + 1)])
            nc.vector.tensor_copy(sdrow[:, 512 * g:512 * (g + 1)],
                                  srow_ps[:])
            bc_ps = bcp.tile([P, 512], f32, tag="bc", name="bc_ps")
            nc.tensor.matmul(bc_ps[:],
                             ones_row_bf[:], sdrow[0:1, 512 * g:512 * (g + 1)])
            seng = nc.vector if g % 2 == 0 else nc.scalar
            any_copy(seng, s_rep[:, 512 * g:512 * (g + 1)], bc_ps[:])

            # projection + R construction for this group's 4 blocks
            is_a = mask_eng[4 * g] == "a"
            prj = sps.tile([P, 4 * 66], f32, tag="pj", name="prj")
            prj_v = prj.rearrange("p (b c) -> p b c", c=66)
            for k in range(4):
                b = 4 * g + k
                nc.tensor.matmul(prj[:, 66 * k:66 * (k + 1)],
                                 gT16[:, P * b:P * (b + 1)], w16[:])
            bsl = slice(4 * g, 4 * g + 4)
            d_src = prj_v[:, :, 65]
            s_src = prj_v[:, :, 64]
            nc.vector.tensor_copy(scol[:, bsl], s_src)
            nc.vector.tensor_copy(dcol[:, bsl], d_src)
            nc.vector.tensor_scalar(negd[:, bsl], d_src, -1.0, None,
                                    op0=Alu.mult)
            # v = e^d, v2 = e^{0.2 d} (x1/2 for +-1 blocks)
            bias = negln2[:] if is_a else 0.0
            nc.scalar.activation(vcol[:, bsl], dcol[:, bsl], Act.Exp,
                                 bias=bias)
            nc.scalar.activation(v2col[:, bsl], dcol[:, bsl], Act.Exp,
                                 scale=0.2, bias=bias)
            for k in range(4):
                b = 4 * g + k
                r0 = RC * b
                nc.vector.tensor_scalar(r_all[:, r0:r0 + F], prj_v[:, k, 0:F],
                                        vcol[:, b:b + 1], None, op0=Alu.mult)
                nc.vector.tensor_tensor(
                    r_all[:, r0 + F + 1:r0 + 2 * F + 1],
                    prj_v[:, k, 0:F],
                    v2col[:, b:b + 1].to_broadcast([P, F]),
                    op=Alu.mult)
            nc.gpsimd.tensor_copy(r_v[:, bsl, F], vcol[:, bsl])
            nc.gpsimd.tensor_copy(r_v[:, bsl, 129], v2col[:, bsl])
            # batch-0 masks + sup-0 accumulation for this group's blocks
            acc0 = get_acc(0)
            for k in range(4):
                b = 4 * g + k
                emit_mask(0, b)
                for t in range(ISUP):
                    nc.tensor.matmul(acc0[:, 512 * t:512 * t + RC],
                                     mask_tiles[(0, b)][:, t * P:(t + 1) * P],
                                     r_v[:, b, 0:RC],
                                     start=(b == chain0[0]),
                                     stop=(b == chain0[-1] and GA == 0))

        # ---- rcol + column-total bursts + broadcast of [C | T2true] ----
        nc.scalar.activation(rcol[:], scol[:], Act.Exp, scale=0.8)
        vp_blocks = [b for b in range(NB) if mask_eng[b] != "a"]
        a_blocks = [b for b in range(NB) if mask_eng[b] == "a"]
        bd_ps = sps.tile([1, RC], f32, tag="pj", name="bd_ps")
        for i, b in enumerate(vp_blocks):
            nc.tensor.matmul(bd_ps[:], ones_col_bf[:], r_v[:, b, 0:RC],
                             start=(i == 0), stop=(i == len(vp_blocks) - 1))
        if a_blocks:
            ba_ps = sps.tile([1, RC], f32, tag="pj", name="ba_ps")
            for i, b in enumerate(a_blocks):
                nc.tensor.matmul(ba_ps[:], ones_col_bf[:], r_v[:, b, 0:RC],
                                 start=(i == 0), stop=(i == len(a_blocks) - 1))
            # C -> bf16 hi/lo rows for the per-bank K=2 correction matmul
            # (row 1 is written via a tiny SBUF->SBUF DMA: engines cannot
            # address a tile starting at partition 1)
            nc.vector.tensor_copy(chl[0:1, :], ba_ps[:])
            nc.vector.tensor_tensor(clo_f[:], ba_ps[:], chl[0:1, :],
                                    op=Alu.subtract)
            nc.vector.tensor_copy(clo_bf[:], clo_f[:])
            nc.sync.dma_start(chl[1:2, :], clo_bf[:])
            bdt2 = persist.tile([1, 65], f32)
            nc.vector.tensor_copy(bdt2[:], bd_ps[:, F + 1:RC])
            nc.vector.scalar_tensor_tensor(cacc[:], ba_ps[:, F + 1:RC],
                                           2.0, bdt2[:],
                                           op0=Alu.mult, op1=Alu.add)
        else:
            nc.vector.tensor_copy(cacc[:], bd_ps[:, F + 1:RC])
        crep_ps = sps.tile([P, 65], f32, tag="pj", name="crep_ps")
        nc.tensor.matmul(crep_ps[:], ones_row_f[:], cacc[:])
        nc.scalar.copy(crep[:], crep_ps[:])

        # Striped block order for mask emission + chain consumption: each
        # sup's emission quota mixes DVE and ACT blocks proportionally so
        # both engines generate masks concurrently instead of in phases.
        vblocks = [b for b in range(NB) if mask_eng[b] == "v"]
        pblocks = [b for b in range(NB) if mask_eng[b] == "p"]
        ablocks = [b for b in range(NB) if mask_eng[b] == "a"]
        ORDER = []
        for h in range(SB):
            for cls in (vblocks, pblocks, ablocks):
                n0 = len(cls) * h // SB
                n1 = len(cls) * (h + 1) // SB
                ORDER.extend(cls[n0:n1])
        assert sorted(ORDER) == list(range(NB))

        # ---- epilogue ----
        late_pend = []

        def late_phase(lsup):
            """num/den algebra + out DMA for i-blocks 2*lsup, 2*lsup+1.
            eps[0:65] is already r-scaled by the drain; everything here is
            a plain Pool tensor_tensor (ACT/DVE stay free for masks)."""
            i0 = ISUP * lsup
            # the final sups' epilogue rides the by-then-idle DVE instead
            # of waiting out Pool's queue
            te = nc.vector if lsup >= NSUP - 2 else nc.gpsimd
            dd = ep.tile([P, ISUP], f32, tag="dd", name="dd")
            te.tensor_tensor(dd[:],
                             crep[:, 64:65].to_broadcast([P, ISUP]),
                             eps_v[:, i0:i0 + ISUP, 129], op=Alu.subtract)
            den = ep.tile([P, ISUP], f32, tag="den", name="den")
            te.tensor_tensor(den[:], eps_v[:, i0:i0 + ISUP, F], dd[:],
                             op=Alu.add)
            rden = ep.tile([P, ISUP], f32, tag="rden", name="rden")
            nc.vector.reciprocal(rden[:], den[:])
            ost = osp.tile([P, ISUP * F], f32, tag="ost", name="ost")
            for t in range(ISUP):
                i = i0 + t
                e0 = RC * i
                B = ep.tile([P, F], f32, tag="nB", name="nB")
                te.tensor_tensor(B[:], crep[:, 0:F],
                                 eps_all[:, e0 + F + 1:e0 + 2 * F + 1],
                                 op=Alu.subtract)
                num = ep.tile([P, F], f32, tag="num", name="num")
                te.tensor_tensor(num[:], eps_all[:, e0:e0 + F], B[:],
                                 op=Alu.add)
                te.tensor_tensor(ost[:, F * t:F * (t + 1)], num[:],
                                 rden[:, t:t + 1].to_broadcast([P, F]),
                                 op=Alu.mult)
            nc.sync.dma_start(
                out_d.rearrange("(c p) f -> p c f", p=P)[:, ISUP * lsup:ISUP * (lsup + 1), :],
                ost.rearrange("p (c f) -> p c f", f=F))

        # ---- main loop over sups (sup 0's matmuls were emitted in setup) ----
        for sup in range(NSUP):
            batch = sup // SB
            nb_ = batch + 1
            if nb_ < NBATCH:
                per = NB // SB
                half = sup % SB
                for b in ORDER[per * half:per * (half + 1)]:
                    emit_mask(nb_, b)
            acc = get_acc(sup)
            if sup > 0:
                off = (sup % SB) * ISUP * P
                for bi, b in enumerate(ORDER):
                    mt = mask_tiles[(batch, b)]
                    for t in range(ISUP):
                        nc.tensor.matmul(acc[:, 512 * t:512 * t + RC],
                                         mt[:, off + t * P:off + (t + 1) * P],
                                         r_v[:, b, 0:RC],
                                         start=(bi == 0),
                                         stop=(bi == NB - 1 and GA == 0))
            if GA > 0:
                # add the +-1-block column correction C into each bank
                for t in range(ISUP):
                    nc.tensor.matmul(acc[:, 512 * t:512 * t + RC],
                                     ones2[:], chl[:],
                                     start=False, stop=True)
            if sup % SB == SB - 1:
                for b in range(NB):
                    mask_tiles.pop((batch, b))
            for t in range(ISUP):
                i = sup * ISUP + t
                # r-scale R1|v during the drain; plain-copy R2|v2.
                # Alternate whole drains DVE/ACT by bank so the two
                # engines' PSUM reads hit different banks.
                if t == 0:
                    nc.vector.tensor_scalar(eps_all[:, RC * i:RC * i + F + 1],
                                            acc[:, 512 * t:512 * t + F + 1],
                                            rcol[:, i:i + 1], None,
                                            op0=Alu.mult)
                    nc.vector.tensor_copy(
                        eps_all[:, RC * i + F + 1:RC * (i + 1)],
                        acc[:, 512 * t + F + 1:512 * t + RC])
                else:
                    nc.scalar.mul(eps_all[:, RC * i:RC * i + F + 1],
                                  acc[:, 512 * t:512 * t + F + 1],
                                  rcol[:, i:i + 1])
                    nc.scalar.copy(eps_all[:, RC * i + F + 1:RC * (i + 1)],
                                   acc[:, 512 * t + F + 1:512 * t + RC])
            late_pend.append(sup)
            if len(late_pend) > 1:
                late_phase(late_pend.pop(0))
        for pr in late_pend:
            late_phase(pr)

    nc.compile()
    return nc


def _get_nc():
    if "nc" not in _CACHE:
        _CACHE["nc"] = _build()
    return _CACHE["nc"]


def kernel(graph, W, a):
    from concourse.bass_utils import run_bass_kernel_spmd

    graph = np.asarray(graph, dtype=np.float32)
    W = np.asarray(W, dtype=np.float32)
    a = np.asarray(a, dtype=np.float32)
    gt = np.ascontiguousarray(graph.T)

    nc = _get_nc()
    in_maps = [
        {
            "gt": gt,
            "w": np.ascontiguousarray(W[h]),
            "wt": np.ascontiguousarray(W[h].T),
            "a2": np.ascontiguousarray(a[h].reshape(2, F).T),
        }
        for h in range(H)
    ]
    trace = bool(int(os.environ.get("GAT_TRACE", "0")))
    res = run_bass_kernel_spmd(nc, in_maps, core_ids=list(range(H)), trace=trace)
    _CACHE["last_result"] = res
    return np.stack([res.results[h]["out"] for h in range(H)], axis=0)
